# revision 2
# baseline (speedup 1.0000x reference)
"""Paged attention (decode) on 8 Trainium2 NeuronCores.

Sharding: tensor-parallel over KV heads — core h owns kv head h, its 4
query heads, and the per-head slices of both caches.

Per-core algorithm (all layouts chosen so softmax runs with tokens on
partitions and no on-chip transposes are needed):

  host prep:
    K cache head-slice  -> 8-token rows [t'(8) x d(128)] bf16
    V cache head-slice  -> 8-token rows [t'(8) x d(128)] bf16
    Q head-slice        -> QT [d, seq, 4] bf16
    block tables        -> per-seq row-index lists (int16), padded to 128
    masks               -> 0/1 validity for each seq's last 1024-token group

  device, per sequence:
    dma_gather(transpose=True)  on K rows -> KT tiles [d=128, t'(8), idx]
    dma_gather(transpose=False) on V rows -> V tiles [row=128, grp, t'*d]
      (V gather sized to the exact row count; the padded tail of the last
       128-row group is never touched — PV contracts only valid rows)
    QK:  S chunk [row=128, 4] = K_slab.T @ Q   (1 matmul per (chunk, t'))
    exp(scale*S) + multiplicative mask on the last group -> P bf16
    PV:  out [d=128, q=4] += V_slab.T-contracted-over-rows @ P slab
    denom: ones[128,1].T @ P -> [1, 4, ct] -> reduced to [1, q]
  epilogue: replicate denominators across partitions with a 1-partition
  matmul, reciprocal, scale, DMA out^T [d, seq*4+q] to DRAM.
"""

import numpy as np
import ml_dtypes

import concourse.bass as bass
import concourse.mybir as mybir
import concourse.tile as tile
from concourse import bacc
from concourse.bass_utils import run_bass_kernel_spmd

BF16 = ml_dtypes.bfloat16
BLOCK_SIZE = 16
ROWS_PER_BLOCK = 2          # 8-token rows
TOK_PER_ROW = 8
ROW_ELEMS = TOK_PER_ROW * 128


def _build_program(n_blocks, n_seqs, n_list, v_list, offs, nslot, repeat=1):
    """Build the (single, SPMD) Bass program.

    n_list[s]: padded row count for seq s (multiple of 128)
    v_list[s]: exact row count for seq s (multiple of 16, <= n_list[s])
    offs[s]:   int16-slot offset of seq s's indices (units of 16 idxs)
    nslot:     total idx slots (free dim of the idx tensor)
    """
    D = 128
    nrows_total = n_blocks * ROWS_PER_BLOCK
    f32 = mybir.dt.float32
    bf16 = mybir.dt.bfloat16

    nc = bacc.Bacc("TRN2", target_bir_lowering=False, debug=False)
    k_d = nc.dram_tensor("krows", [nrows_total, ROW_ELEMS], bf16, kind="ExternalInput")
    v_d = nc.dram_tensor("vrows", [nrows_total, ROW_ELEMS], bf16, kind="ExternalInput")
    q_d = nc.dram_tensor("qt", [D, n_seqs, 4], bf16, kind="ExternalInput")
    idx_d = nc.dram_tensor("idx", [128, nslot], mybir.dt.int16, kind="ExternalInput")
    m_d = nc.dram_tensor("masks", [128, n_seqs, TOK_PER_ROW, 4], bf16,
                         kind="ExternalInput")
    o_d = nc.dram_tensor("out", [D, n_seqs * 4], f32, kind="ExternalOutput")

    with tile.TileContext(nc) as tc:
        with (
            tc.tile_pool(name="const", bufs=1) as const,
            tc.tile_pool(name="kp", bufs=4) as kp,
            tc.tile_pool(name="vp", bufs=4) as vp,
            tc.tile_pool(name="pp", bufs=2) as pp,
            tc.tile_pool(name="psS", bufs=2, space="PSUM") as psSp,
            tc.tile_pool(name="psO", bufs=1, space="PSUM") as psOp,
            tc.tile_pool(name="psD", bufs=2, space="PSUM") as psDp,
            tc.tile_pool(name="psR", bufs=1, space="PSUM") as psRp,
        ):
            qt = const.tile([D, n_seqs, 4], bf16)
            idx_t = const.tile([128, nslot], mybir.dt.int16)
            masks = const.tile([128, n_seqs, TOK_PER_ROW, 4], bf16)
            ones = const.tile([128, 1], bf16)
            onesr = const.tile([1, 128], f32)
            dsb = const.tile([1, n_seqs, 4], f32)
            outsb = const.tile([D, n_seqs * 4], f32)
            recs = const.tile([D, n_seqs * 4], f32)
            nc.sync.dma_start(qt[:], q_d[:])
            nc.sync.dma_start(idx_t[:], idx_d[:])
            nc.sync.dma_start(masks[:], m_d[:])
            nc.vector.memset(ones[:], 1.0)
            nc.vector.memset(onesr[:], 1.0)

            psO = psOp.tile([D, n_seqs * 4], f32)

            scale = float(1.0 / np.sqrt(np.float32(D)).astype(np.float32))

            for s in [s for _ in range(repeat) for s in range(n_seqs)]:
                n = n_list[s]          # padded rows (mult of 128)
                nv = v_list[s]         # exact rows (mult of 16)
                C = n // 128
                CV = -(-nv // 128)     # V groups present (partial last)

                k_t = kp.tile([D, TOK_PER_ROW, n], bf16, tag="k")
                v_t = vp.tile([128, C, ROW_ELEMS], bf16, tag="v")
                i_ap = idx_t[:, offs[s]:offs[s] + n // 16]
                iv_ap = idx_t[:, offs[s]:offs[s] + nv // 16]
                nc.gpsimd.dma_gather(k_t[:], k_d[:], i_ap, n, n, ROW_ELEMS,
                                     transpose=True)
                nc.gpsimd.dma_gather(v_t[:, 0:CV, :], v_d[:], iv_ap, nv, nv,
                                     ROW_ELEMS)

                psS = psSp.tile([128, C * TOK_PER_ROW, 4], f32, tag="psS")
                rhs_q = qt[:, s, :]
                for c in range(C):
                    for tp in range(TOK_PER_ROW):
                        nc.tensor.matmul(psS[:, c * TOK_PER_ROW + tp, :],
                                         k_t[:, tp, c * 128:(c + 1) * 128],
                                         rhs_q, start=True, stop=True)

                p_t = pp.tile([128, C * TOK_PER_ROW, 4], bf16, tag="p")
                nc.scalar.activation(p_t[:], psS[:],
                                     mybir.ActivationFunctionType.Exp,
                                     scale=scale)
                nc.vector.tensor_mul(
                    p_t[:, (C - 1) * TOK_PER_ROW:C * TOK_PER_ROW, :],
                    p_t[:, (C - 1) * TOK_PER_ROW:C * TOK_PER_ROW, :],
                    masks[:, s, :, :])

                nmm = CV * TOK_PER_ROW
                i_mm = 0
                for c in range(CV):
                    r = min(128, nv - c * 128)   # valid rows this group
                    for tp in range(TOK_PER_ROW):
                        nc.tensor.matmul(
                            psO[:, s * 4:(s + 1) * 4],
                            v_t[0:r, c, tp * 128:(tp + 1) * 128],
                            p_t[0:r, c * TOK_PER_ROW + tp, :],
                            start=(i_mm == 0), stop=(i_mm == nmm - 1))
                        i_mm += 1

                psD = psDp.tile([1, 4, C * TOK_PER_ROW], f32, tag="psD")
                nc.tensor.matmul(psD[:], ones[:],
                                 p_t[:].transpose([0, 2, 1]),
                                 start=True, stop=True)
                nc.vector.tensor_reduce(dsb[:, s, :], psD[:],
                                        mybir.AxisListType.X,
                                        mybir.AluOpType.add)

            # epilogue: replicate denominators to all partitions, divide
            psR = psRp.tile([128, n_seqs * 4], f32)
            nc.tensor.matmul(psR[:], onesr[:], dsb[0:1, :, :],
                             start=True, stop=True)
            nc.vector.reciprocal(recs[:], psR[:])
            nc.vector.tensor_mul(outsb[:], psO[:], recs[:])
            nc.sync.dma_start(o_d[:], outsb[:])

    nc.compile()
    return nc


def prepare(query, key_cache, value_cache, block_tables, context_lens,
            repeat=1):
    """Host prep + program build. Returns (nc, in_maps, (B, H, D, kvh))."""
    query = np.asarray(query)
    key_cache = np.asarray(key_cache)
    value_cache = np.asarray(value_cache)
    block_tables = np.asarray(block_tables)
    context_lens = np.asarray(context_lens)

    nb_tot, kvh, dx, bs, x = key_cache.shape
    D = dx * x
    B, H, _ = query.shape
    qpk = H // kvh
    assert D == 128 and bs == BLOCK_SIZE and qpk == 4 and B * 4 <= 128

    ctx = context_lens.astype(np.int64)
    nb = -(-ctx // BLOCK_SIZE)                       # blocks per seq
    nrows = ROWS_PER_BLOCK * nb                      # 8-token rows per seq
    v_list = (-(-nrows // 16) * 16).astype(np.int64)    # exact, mult of 16
    n_list = (-(-nrows // 128) * 128).astype(np.int64)  # padded to 128
    C_list = n_list // 128
    offs = np.zeros(B, dtype=np.int64)
    acc = 0
    for s in range(B):
        offs[s] = acc
        acc += n_list[s] // 16
    nslot = int(acc)

    # ---- shared (head-independent) host prep ----
    # per-seq index lists (idx-0 padding keeps gathered garbage finite)
    idx_flat = np.zeros(nslot * 16, dtype=np.int16)
    for s in range(B):
        m = int(nrows[s])
        rows = (block_tables[s, :nb[s], None] * ROWS_PER_BLOCK
                + np.arange(ROWS_PER_BLOCK)[None, :]).reshape(-1).astype(np.int16)
        base = offs[s] * 16
        idx_flat[base:base + m] = rows
    idx_wrapped = np.ascontiguousarray(
        idx_flat.reshape(nslot, 16).T)               # [16, nslot]
    idx_rep = np.ascontiguousarray(np.tile(idx_wrapped, (8, 1)))  # [128, nslot]

    # masks [128, B, 8, 4]: validity of each seq's LAST 1024-token group
    p_ar = np.arange(128)
    tp_ar = np.arange(TOK_PER_ROW)
    L = TOK_PER_ROW * p_ar[:, None] + tp_ar[None, :]
    masks = np.zeros((128, B, TOK_PER_ROW, 4), dtype=np.float32)
    for s in range(B):
        rem = int(ctx[s]) - 128 * TOK_PER_ROW * (int(C_list[s]) - 1)
        masks[:, s, :, 0] = (L < rem).astype(np.float32)
    masks[:, :, :, 1:] = masks[:, :, :, 0:1]
    masks = masks.astype(BF16)

    # ---- per-core prep ----
    in_maps = []
    for h in range(kvh):
        kc = key_cache[:, h]                          # [NB, dx, bs, x]
        K = np.ascontiguousarray(kc.transpose(0, 2, 1, 3)).reshape(nb_tot, bs, D)
        krows = K.reshape(nb_tot * ROWS_PER_BLOCK, ROW_ELEMS).astype(BF16)

        vc = value_cache[:, h]                        # [NB, D, bs]
        V = np.ascontiguousarray(vc.transpose(0, 2, 1))  # [NB, bs, D]
        vrows = V.reshape(nb_tot * ROWS_PER_BLOCK, ROW_ELEMS).astype(BF16)

        qh = query[:, 4 * h:4 * h + 4, :]             # [B, 4, D]
        qt = np.ascontiguousarray(qh.transpose(2, 0, 1)).astype(BF16)  # [D, B, 4]

        in_maps.append({
            "krows": krows, "vrows": vrows,
            "qt": qt, "idx": idx_rep,
            "masks": masks,
        })

    build_args = (nb_tot, B, [int(v) for v in n_list],
                  [int(v) for v in v_list],
                  [int(v) for v in offs], nslot)
    globals()["_last_build_args"] = build_args
    nc = _build_program(*build_args, repeat=repeat)
    return nc, in_maps, (B, H, D, kvh)


def assemble(res, meta):
    B, H, D, kvh = meta
    out = np.empty((B, H, D), dtype=np.float32)
    for h in range(kvh):
        o = res[h]["out"]                             # [D, B*4]
        out[:, 4 * h:4 * h + 4, :] = o.reshape(D, B, 4).transpose(1, 2, 0)
    return out


def kernel(query, key_cache, value_cache, block_tables, context_lens):
    nc, in_maps, meta = prepare(query, key_cache, value_cache,
                                block_tables, context_lens)
    kres = run_bass_kernel_spmd(nc, in_maps, list(range(meta[3])))
    globals()["_last_results"] = kres
    return assemble(kres.results, meta)
